# revision 35
# baseline (speedup 1.0000x reference)
"""BEV-pool (segment-sum scatter) Trainium2 kernel for nn_BaseDepthTransform.

Design (v2 — canonical-slot packing, constant one-hot, 3 PE column groups):

  Host (numpy): replicate the reference geometry -> per-point flat BEV bin id
  (depends only on the small camera matrices, not on x). Bins are sorted by
  point count (desc) and grouped 16-per-WINDOW so bins sharing a window have
  similar counts. Every 128-lane tile uses one CANONICAL lane->slot map:
  lane p holds points of window-slot p//8, two points per lane (A half in
  cols 0:80, B half in 80:160). A window with max-bin-count c needs
  L = ceil(c/16) tiles; L is binary-decomposed into class {4,2} chain
  segments. The one-hot lhsT is therefore a single CONSTANT [128,16] tile
  shared by every matmul — no per-tile lid stream, no on-device one-hot
  build, and the first matmul only waits on its own small feats sub-DMA.

  Device (Bass/Tile, SPMD x8): stream of WAVES of 27 segments
  (3 thirds t x 3 PE column groups g x 3 slots s; stream position
  p = t*9 + g*3 + s). Per THIRD t one feats sub-DMA [128, 9*c*160] fp8
  on the sync HW-DGE ring (measured best: finer 9-per-wave splits and
  scalar-/gpsimd-ring variants all regressed — the 8 global HWDGE
  semaphore lanes pace triggers at sub-DMA-completion rate either way,
  and extra triggers/rings only add overhead). Wave 0's thirds trigger
  from sync/scalar/gpsimd in parallel to cut startup serialization.
  Matmul chains accumulate [16,160] per segment into PSUM; chains are
  emitted as triples across the 3 column groups with pairwise-distinct
  PSUM banks (same-bank interleaved open chains corrupt; bank(g,t) =
  (g+t)%3 keeps each concurrent triple on 3 distinct banks while each
  column group still touches all 9 (bank,slot) cells per wave).
  PSUM->SBUF copy in bf16: ONE vector-engine op [80,1440] per wave,
  bank-major strided (engine cost scales with free size only; dead
  partition rows 16-31/48-63 copy garbage the host ignores;
  per-colgroup ops get serialized by the scheduler's transitive dep
  chaining and cost 3x). ONE gpsimd DMA per wave ships st[0:80] to
  out[80, .] (dead rows included — 10 tail triggers instead of 30
  beats the extra write bytes).

  Wave order: one class-2 wave first (small first DMA -> fast start), all
  class-4 waves (bulk), remaining class-2 waves last (short drain tail).

  Host reassembly: per segment, grid[window bins] += out[:, :80] + out[:, 80:].
"""
import sys
sys.path.insert(0, '/opt/trn_rl_repo')

import numpy as np
import ml_dtypes

FP8E3 = ml_dtypes.float8_e3m4

# ---- static problem config (mirrors the reference) ----
IH, IW = 256, 704
FH, FW = 32, 88
D = 118
C = 80
C2 = 2 * C
NXg, NYg, NZg = 360, 360, 1
BXc = np.array([-53.85, -53.85, 0.0], np.float32)
DXc = np.array([0.3, 0.3, 20.0], np.float32)
NBINS = NZg * NXg * NYg  # 129600
W = 16                    # bins per window (= one-hot width = PSUM partitions)
PTS_PER_SLOT = 16         # 8 lanes x 2 points per bin-slot per tile
NCORES = 8
CLASSES = (4, 2)          # chain segment lengths (binary decomposition)
NCG = 3                   # PE column groups (PSUM partition bases 0,32,64)
NBANK = 3                 # PSUM banks per wave
NSLOT = 3                 # segment slots per bank (3*160 = 480 of 512 words)
WAVE = NCG * NBANK * NSLOT  # 27 segments per wave
STC = NBANK * NSLOT * C2    # staging cols per column group per wave (1440)

_BUILD_CACHE = {}


def _frustum():
    ds = np.arange(1.0, 60.0, 0.5, dtype=np.float32)
    xs = np.linspace(0.0, IW - 1.0, FW, dtype=np.float32)
    ys = np.linspace(0.0, IH - 1.0, FH, dtype=np.float32)
    ds_g = np.broadcast_to(ds[:, None, None], (D, FH, FW))
    xs_g = np.broadcast_to(xs[None, None, :], (D, FH, FW))
    ys_g = np.broadcast_to(ys[None, :, None], (D, FH, FW))
    return np.stack([xs_g, ys_g, ds_g], axis=-1)  # [D,FH,FW,3]


def _get_geometry(c2l_rots, c2l_trans, intrins, post_rots, post_trans,
                  extra_rots, extra_trans):
    fr = _frustum()
    pts = fr[None, None] - post_trans[:, :, None, None, None, :]
    inv_pr = np.linalg.inv(post_rots).astype(np.float32)
    pts = np.einsum('bnij,bndhwj->bndhwi', inv_pr, pts).astype(np.float32)
    pts = np.concatenate([pts[..., :2] * pts[..., 2:3], pts[..., 2:3]], axis=-1)
    combine = np.einsum(
        'bnij,bnjk->bnik', c2l_rots, np.linalg.inv(intrins).astype(np.float32)
    ).astype(np.float32)
    pts = np.einsum('bnij,bndhwj->bndhwi', combine, pts).astype(np.float32)
    pts = pts + c2l_trans[:, :, None, None, None, :]
    pts = np.einsum('bij,bndhwj->bndhwi', extra_rots, pts).astype(np.float32)
    pts = pts + extra_trans[:, None, None, None, None, :]
    return pts  # [B,N,D,FH,FW,3]


def _flat_bins(geom):
    """Per-point flat bin id (int64), -1 for dropped points."""
    coords = ((geom - (BXc - DXc / 2.0)) / DXc).astype(np.int32)
    B = coords.shape[0]
    coords = coords.reshape(B, -1, 3)
    cx, cy, cz = coords[..., 0], coords[..., 1], coords[..., 2]
    kept = (cx >= 0) & (cx < NXg) & (cy >= 0) & (cy < NYg) & (cz >= 0) & (cz < NZg)
    flat = ((cz.astype(np.int64) * NXg + cx) * NYg + cy)
    flat = np.where(kept, flat, -1)
    return flat  # [B, Np]


def _plan(flat):
    """Canonical-slot schedule from bin ids. Returns dict with:
    per-point scatter coords, per-class per-core segment window ids, shapes."""
    kept_idx = np.nonzero(flat >= 0)[0]
    fk = flat[kept_idx]
    order = np.argsort(fk, kind='stable')
    fks = fk[order]
    pidx = kept_idx[order]                       # point index per sorted pt
    ub, inv, cnts = np.unique(fks, return_inverse=True, return_counts=True)
    nb = len(ub)
    # rank bins by count desc (stable -> deterministic)
    rank_of_bin = np.empty(nb, np.int64)
    by_cnt = np.argsort(-cnts, kind='stable')
    rank_of_bin[by_cnt] = np.arange(nb)
    nwin = (nb + W - 1) // W
    # per-window tile count L
    cnt_pad = np.r_[cnts[by_cnt], np.zeros(nwin * W - nb, np.int64)]
    L = np.ceil(cnt_pad.reshape(nwin, W).max(axis=1) / PTS_PER_SLOT).astype(np.int64)
    tile_base = np.r_[0, np.cumsum(L)]
    # per-point coords
    run_start = np.r_[0, np.cumsum(cnts)[:-1]]
    r = np.arange(len(fks)) - run_start[inv]     # rank within bin
    rk = rank_of_bin[inv]                        # global bin rank
    win = rk // W
    slot = rk % W
    layer = r // PTS_PER_SLOT
    within = r % PTS_PER_SLOT
    lane = slot * 8 + within // 2
    half = within % 2
    gtile = tile_base[win] + layer
    # window bins for reassembly: bin id at (window, slot)
    win_bins = np.full(nwin * W, -1, np.int64)
    win_bins[:nb] = ub[by_cnt]
    win_bins = win_bins.reshape(nwin, W)
    # class segments: (window, tile_start) in window order
    segs = {c: [] for c in CLASSES}
    Lrem = L.copy()
    start = tile_base[:-1].copy()
    for c in CLASSES:
        ns = Lrem // c
        for w in np.nonzero(ns)[0]:
            for k in range(ns[w]):
                segs[c].append((w, start[w] + k * c))
        start = start + ns * c
        Lrem = Lrem - ns * c
    assert (Lrem == 0).all()
    return {
        "pidx": pidx, "gtile": gtile, "lane": lane, "half": half,
        "ntiles": int(tile_base[-1]), "win_bins": win_bins, "segs": segs,
    }


def _core_split(segs):
    """Per class: contiguous split across cores; returns per-core lists and
    padded Gmax."""
    out = {}
    for c in CLASSES:
        cl = segs[c]
        G = len(cl)
        per = []
        for ci in range(NCORES):
            lo = (G * ci) // NCORES
            hi = (G * (ci + 1)) // NCORES
            per.append(cl[lo:hi])
        Gmax = max(1, max(len(p) for p in per))
        out[c] = (per, Gmax)
    return out


def _build_core_inputs(split, feats_all):
    """Per-core input dict: per class wave-ordered feats streams + one-hot."""
    maps = [dict() for _ in range(NCORES)]
    meta = {c: [] for c in CLASSES}
    zrow = feats_all.shape[0] - 1                # zero sentinel tile
    for c in CLASSES:
        per, Gmax = split[c]
        for ci in range(NCORES):
            segs = per[ci]
            tid = np.full((Gmax, c), zrow, np.int64)
            wins = np.full(Gmax, -1, np.int64)
            for j, (w, t0) in enumerate(segs):
                tid[j] = np.arange(t0, t0 + c)
                wins[j] = w
            fc = feats_all[tid.reshape(-1)]       # [Gmax*c, 128, C2]
            fc = np.ascontiguousarray(
                fc.transpose(1, 0, 2).reshape(128, Gmax * c * C2))
            maps[ci][f"feats{c}"] = fc
            meta[c].append(wins)
    onehot = (np.arange(128)[:, None] // 8 == np.arange(W)[None, :])
    oh = np.ascontiguousarray(onehot.astype(FP8E3))
    for ci in range(NCORES):
        maps[ci]["onehot"] = oh
    return maps, meta


def _wave_plan(Gmax_by_class):
    """Emission order: one class-2 wave first, all class-4, rest of class-2.
    Returns list of (class, wave_idx, nseg_in_wave) and per-class wave count."""
    nwv = {c: (Gmax_by_class[c] + WAVE - 1) // WAVE for c in CLASSES}
    order = []

    def cw(c, w):
        ns = min(WAVE, Gmax_by_class[c] - w * WAVE)
        order.append((c, w, ns))

    if nwv[2] > 0:
        cw(2, 0)
    for w in range(nwv[4]):
        cw(4, w)
    for w in range(1, nwv[2]):
        cw(2, w)
    return order, nwv


def _build_bass(shape_key):
    """shape_key: tuple of (cls, Gmax) pairs, CLASSES order."""
    if shape_key in _BUILD_CACHE:
        return _BUILD_CACHE[shape_key]
    from concourse import bass, mybir, tile, bacc

    Gmax_by_class = dict(shape_key)
    nc = bacc.Bacc()
    params = {}
    for c, Gmax in shape_key:
        params[f"feats{c}"] = nc.declare_dram_parameter(
            f"feats{c}", [128, Gmax * c * C2], mybir.dt.float8e3,
            isOutput=False)
    order, nwv = _wave_plan(Gmax_by_class)
    for c in CLASSES:
        if nwv[c]:
            params[f"out{c}"] = nc.declare_dram_parameter(
                f"out{c}", [80, nwv[c] * STC], mybir.dt.bfloat16,
                isOutput=True)
    params["onehot"] = nc.declare_dram_parameter(
        "onehot", [128, W], mybir.dt.float8e3, isOutput=False)
    FW_COLS = 9 * CLASSES[0] * C2       # fixed sub-DMA tile size (class-4)

    with tile.TileContext(nc) as tc:
        with tc.tile_pool(name="fstream", bufs=9) as fpool, \
             tc.tile_pool(name="stage", bufs=4) as spool, \
             tc.tile_pool(name="const", bufs=1) as cpool, \
             tc.tile_pool(name="psum", bufs=2, space="PSUM") as psum_pool:
            oh_t = cpool.tile([128, W], mybir.dt.float8e3, tag="oh")
            nc.sync.dma_start(oh_t[:], params["onehot"][:, :])
            for wi, (c, wv, NW) in enumerate(order):
                # sub-DMA per third (s-major layout). Thirds 0-1 on the
                # sync HW-DGE ring, third 2 on gpsimd's SWDGE ring so the
                # two descriptor rings pace independently (the 8 HWDGE
                # semaphore lanes otherwise serialize trigger pacing).
                # per-third sub-DMAs on the sync ring (measured optimum:
                # whole-wave DMAs regress — engines fair-share across
                # outstanding transfers so big DMAs complete late; 9/wave
                # regresses on trigger overhead). Wave 0's thirds go out
                # on three parallel queues for a fast first MM.
                fts = []
                for t in range(NBANK):
                    ng = max(0, min(9, NW - t * 9))
                    ft = fpool.tile([128, 9 * CLASSES[0] * C2],
                                    mybir.dt.float8e3, tag=f"f{t}")
                    if ng:
                        a = (wv * WAVE + t * 9) * c * C2
                        eng = (nc.sync, nc.scalar, nc.gpsimd)[t] \
                            if wi == 0 else nc.sync
                        eng.dma_start(ft[:, :ng * c * C2],
                                      params[f"feats{c}"][:, a:a + ng * c * C2])
                    fts.append((ft, ng))
                mega = psum_pool.tile([80, NBANK * 512], mybir.dt.float32,
                                      tag="ps")
                # chains: triple-interleaved across column groups,
                # pairwise-distinct banks (bank = (g+t)%3)
                for t in range(NBANK):
                    ft, ng = fts[t]
                    for s in range(NSLOT):
                        for k in range(c):
                            for g in range(NCG):
                                pos = g * NSLOT + s
                                if pos >= ng:
                                    continue
                                b = (g + t) % NBANK
                                col = (pos * c + k) * C2
                                nc.tensor.matmul(
                                    out=mega[g * 32:g * 32 + W,
                                             b * 512 + s * C2:
                                             b * 512 + s * C2 + C2],
                                    lhsT=oh_t[:],
                                    rhs=ft[:, col:col + C2],
                                    start=(k == 0), stop=(k == c - 1))
                # PSUM -> SBUF (bf16), bank-major strided. ONE op covering
                # all 80 partitions (engine cost scales with free size only;
                # dead rows 16-31/48-63 copy garbage the host ignores) —
                # three per-colgroup ops get serialized by the scheduler's
                # transitive dep chaining and cost 3x. Alternate engines.
                st = spool.tile([80, STC], mybir.dt.bfloat16, tag="st")
                sub_m = mega[0:80, :]
                sub_s = st[0:80, :]
                src = bass.AP(sub_m.tensor, sub_m.offset,
                              [sub_m.ap[0], [512, NBANK], [1, NSLOT * C2]])
                dst = bass.AP(sub_s.tensor, sub_s.offset,
                              [sub_s.ap[0], [NSLOT * C2, NBANK],
                               [1, NSLOT * C2]])
                nc.vector.tensor_scalar_add(dst, src, 0.0)
                # ONE out-DMA per wave on gpsimd's SWDGE ring (80 rows
                # incl. dead ones the host ignores): 10 triggers instead
                # of 30 shortens the end-of-stream trigger serialization.
                # Scalar/sync rings for outs measured 6us slower.
                nc.gpsimd.dma_start(
                    params[f"out{c}"][:, wv * STC:(wv + 1) * STC],
                    st[:, :])
    nc.finalize()
    _BUILD_CACHE[shape_key] = nc
    return nc


def run_scheduled(x, flat, trace=False, trace_cores=None):
    """Core pipeline given precomputed flat bins; returns (grid, results)."""
    from concourse.bass_utils import run_bass_kernel_spmd

    plan = _plan(flat)
    xq = np.ascontiguousarray(x.reshape(-1, C)).astype(FP8E3)
    # global canonical tile store (+1 zero sentinel row)
    feats_all = np.zeros((plan["ntiles"] + 1, 128, C2), FP8E3)
    fview = feats_all.reshape(plan["ntiles"] + 1, 128, 2, C)
    fview[plan["gtile"], plan["lane"], plan["half"]] = xq[plan["pidx"]]

    split = _core_split(plan["segs"])
    shape_key = tuple((c, split[c][1]) for c in CLASSES)
    maps, meta = _build_core_inputs(split, feats_all)
    nc = _build_bass(shape_key)
    res = run_bass_kernel_spmd(nc, maps, core_ids=list(range(NCORES)),
                               trace=trace, trace_cores=trace_cores)

    Gmax_by_class = dict(shape_key)
    order, nwv = _wave_plan(Gmax_by_class)
    win_bins = plan["win_bins"]
    grid = np.zeros((NBINS, C), np.float32)
    for c in CLASSES:
        if not nwv[c]:
            continue
        for ci in range(NCORES):
            out = np.asarray(res.results[ci][f"out{c}"], np.float32)
            out = out.reshape(80, nwv[c], NBANK * NSLOT, C2)
            wins = meta[c][ci]
            live = np.nonzero(wins >= 0)[0]
            if not len(live):
                continue
            j = live
            wv, rem = j // WAVE, j % WAVE
            t, gp = rem // 9, rem % 9
            g, s = gp // NSLOT, gp % NSLOT
            blk = ((g + t) % NBANK) * NSLOT + s
            # vals[j] = out[g*32:g*32+16, wv, blk, :]
            rows = (g[:, None] * 32 + np.arange(W)[None, :])  # [nj, 16]
            vals = out[rows, wv[:, None], blk[:, None], :]    # [nj, 16, C2]
            vals = vals[..., :C] + vals[..., C:]              # [nj, 16, C]
            bins = win_bins[wins[j]]                          # [nj, 16]
            m = bins >= 0
            np.add.at(grid, bins[m], vals[m])
    return grid, res


def kernel(x, camera2lidar_rots, camera2lidar_trans, intrins, post_rots,
           post_trans, extra_rots, extra_trans):
    x = np.asarray(x, np.float32)
    B, N = x.shape[0], x.shape[1]
    assert (B, N) == (1, 6) and x.shape[2:] == (D, FH, FW, C), x.shape

    geom = _get_geometry(
        np.asarray(camera2lidar_rots, np.float32),
        np.asarray(camera2lidar_trans, np.float32),
        np.asarray(intrins, np.float32),
        np.asarray(post_rots, np.float32),
        np.asarray(post_trans, np.float32),
        np.asarray(extra_rots, np.float32),
        np.asarray(extra_trans, np.float32),
    )
    flat = _flat_bins(geom)[0]          # [Np]
    grid, _ = run_scheduled(x, flat)
    outp = grid.reshape(NXg, NYg, C).transpose(2, 0, 1)[None]  # [1,C,NX,NY]
    return np.ascontiguousarray(outp)


# revision 36
# speedup vs baseline: 1.0113x; 1.0113x over previous
"""BEV-pool (segment-sum scatter) Trainium2 kernel for nn_BaseDepthTransform.

Design (v2 — canonical-slot packing, constant one-hot, 3 PE column groups):

  Host (numpy): replicate the reference geometry -> per-point flat BEV bin id
  (depends only on the small camera matrices, not on x). Bins are sorted by
  point count (desc) and grouped 16-per-WINDOW so bins sharing a window have
  similar counts. Every 128-lane tile uses one CANONICAL lane->slot map:
  lane p holds points of window-slot p//8, two points per lane (A half in
  cols 0:80, B half in 80:160). A window with max-bin-count c needs
  L = ceil(c/16) tiles; L is binary-decomposed into class {4,2} chain
  segments. The one-hot lhsT is therefore a single CONSTANT [128,16] tile
  shared by every matmul — no per-tile lid stream, no on-device one-hot
  build, and the first matmul only waits on its own small feats sub-DMA.

  Device (Bass/Tile, SPMD x8): stream of WAVES of 27 segments
  (3 thirds t x 3 PE column groups g x 3 slots s; stream position
  p = t*9 + g*3 + s). Per THIRD t one feats sub-DMA [128, 9*c*160] fp8
  on the sync HW-DGE ring (measured best: finer 9-per-wave splits and
  scalar-/gpsimd-ring variants all regressed — the 8 global HWDGE
  semaphore lanes pace triggers at sub-DMA-completion rate either way,
  and extra triggers/rings only add overhead). Wave 0's thirds trigger
  from sync/scalar/gpsimd in parallel to cut startup serialization.
  Matmul chains accumulate [16,160] per segment into PSUM; chains are
  emitted as triples across the 3 column groups with pairwise-distinct
  PSUM banks (same-bank interleaved open chains corrupt; bank(g,t) =
  (g+t)%3 keeps each concurrent triple on 3 distinct banks while each
  column group still touches all 9 (bank,slot) cells per wave).
  PSUM->SBUF copy in bf16: ONE vector-engine op [80,1440] per wave,
  bank-major strided (engine cost scales with free size only; dead
  partition rows 16-31/48-63 copy garbage the host ignores;
  per-colgroup ops get serialized by the scheduler's transitive dep
  chaining and cost 3x). ONE gpsimd DMA per wave ships st[0:80] to
  out[80, .] (dead rows included — 10 tail triggers instead of 30
  beats the extra write bytes).

  Wave order: one class-2 wave first (small first DMA -> fast start), all
  class-4 waves (bulk), remaining class-2 waves last (short drain tail).

  Host reassembly: per segment, grid[window bins] += out[:, :80] + out[:, 80:].
"""
import sys
sys.path.insert(0, '/opt/trn_rl_repo')

import numpy as np
import ml_dtypes

FP8E3 = ml_dtypes.float8_e3m4

# ---- static problem config (mirrors the reference) ----
IH, IW = 256, 704
FH, FW = 32, 88
D = 118
C = 80
C2 = 2 * C
NXg, NYg, NZg = 360, 360, 1
BXc = np.array([-53.85, -53.85, 0.0], np.float32)
DXc = np.array([0.3, 0.3, 20.0], np.float32)
NBINS = NZg * NXg * NYg  # 129600
W = 16                    # bins per window (= one-hot width = PSUM partitions)
PTS_PER_SLOT = 16         # 8 lanes x 2 points per bin-slot per tile
NCORES = 8
CLASSES = (4, 2)          # chain segment lengths (binary decomposition)
NCG = 3                   # PE column groups (PSUM partition bases 0,32,64)
NBANK = 3                 # PSUM banks per wave
NSLOT = 3                 # segment slots per bank (3*160 = 480 of 512 words)
WAVE = NCG * NBANK * NSLOT  # 27 segments per wave
STC = NBANK * NSLOT * C2    # staging cols per column group per wave (1440)

_BUILD_CACHE = {}


def _frustum():
    ds = np.arange(1.0, 60.0, 0.5, dtype=np.float32)
    xs = np.linspace(0.0, IW - 1.0, FW, dtype=np.float32)
    ys = np.linspace(0.0, IH - 1.0, FH, dtype=np.float32)
    ds_g = np.broadcast_to(ds[:, None, None], (D, FH, FW))
    xs_g = np.broadcast_to(xs[None, None, :], (D, FH, FW))
    ys_g = np.broadcast_to(ys[None, :, None], (D, FH, FW))
    return np.stack([xs_g, ys_g, ds_g], axis=-1)  # [D,FH,FW,3]


def _get_geometry(c2l_rots, c2l_trans, intrins, post_rots, post_trans,
                  extra_rots, extra_trans):
    fr = _frustum()
    pts = fr[None, None] - post_trans[:, :, None, None, None, :]
    inv_pr = np.linalg.inv(post_rots).astype(np.float32)
    pts = np.einsum('bnij,bndhwj->bndhwi', inv_pr, pts).astype(np.float32)
    pts = np.concatenate([pts[..., :2] * pts[..., 2:3], pts[..., 2:3]], axis=-1)
    combine = np.einsum(
        'bnij,bnjk->bnik', c2l_rots, np.linalg.inv(intrins).astype(np.float32)
    ).astype(np.float32)
    pts = np.einsum('bnij,bndhwj->bndhwi', combine, pts).astype(np.float32)
    pts = pts + c2l_trans[:, :, None, None, None, :]
    pts = np.einsum('bij,bndhwj->bndhwi', extra_rots, pts).astype(np.float32)
    pts = pts + extra_trans[:, None, None, None, None, :]
    return pts  # [B,N,D,FH,FW,3]


def _flat_bins(geom):
    """Per-point flat bin id (int64), -1 for dropped points."""
    coords = ((geom - (BXc - DXc / 2.0)) / DXc).astype(np.int32)
    B = coords.shape[0]
    coords = coords.reshape(B, -1, 3)
    cx, cy, cz = coords[..., 0], coords[..., 1], coords[..., 2]
    kept = (cx >= 0) & (cx < NXg) & (cy >= 0) & (cy < NYg) & (cz >= 0) & (cz < NZg)
    flat = ((cz.astype(np.int64) * NXg + cx) * NYg + cy)
    flat = np.where(kept, flat, -1)
    return flat  # [B, Np]


def _plan(flat):
    """Canonical-slot schedule from bin ids. Returns dict with:
    per-point scatter coords, per-class per-core segment window ids, shapes."""
    kept_idx = np.nonzero(flat >= 0)[0]
    fk = flat[kept_idx]
    order = np.argsort(fk, kind='stable')
    fks = fk[order]
    pidx = kept_idx[order]                       # point index per sorted pt
    ub, inv, cnts = np.unique(fks, return_inverse=True, return_counts=True)
    nb = len(ub)
    # rank bins by count desc (stable -> deterministic)
    rank_of_bin = np.empty(nb, np.int64)
    by_cnt = np.argsort(-cnts, kind='stable')
    rank_of_bin[by_cnt] = np.arange(nb)
    nwin = (nb + W - 1) // W
    # per-window tile count L
    cnt_pad = np.r_[cnts[by_cnt], np.zeros(nwin * W - nb, np.int64)]
    L = np.ceil(cnt_pad.reshape(nwin, W).max(axis=1) / PTS_PER_SLOT).astype(np.int64)
    tile_base = np.r_[0, np.cumsum(L)]
    # per-point coords
    run_start = np.r_[0, np.cumsum(cnts)[:-1]]
    r = np.arange(len(fks)) - run_start[inv]     # rank within bin
    rk = rank_of_bin[inv]                        # global bin rank
    win = rk // W
    slot = rk % W
    layer = r // PTS_PER_SLOT
    within = r % PTS_PER_SLOT
    lane = slot * 8 + within // 2
    half = within % 2
    gtile = tile_base[win] + layer
    # window bins for reassembly: bin id at (window, slot)
    win_bins = np.full(nwin * W, -1, np.int64)
    win_bins[:nb] = ub[by_cnt]
    win_bins = win_bins.reshape(nwin, W)
    # class segments: (window, tile_start) in window order
    segs = {c: [] for c in CLASSES}
    Lrem = L.copy()
    start = tile_base[:-1].copy()
    for c in CLASSES:
        ns = Lrem // c
        for w in np.nonzero(ns)[0]:
            for k in range(ns[w]):
                segs[c].append((w, start[w] + k * c))
        start = start + ns * c
        Lrem = Lrem - ns * c
    assert (Lrem == 0).all()
    return {
        "pidx": pidx, "gtile": gtile, "lane": lane, "half": half,
        "ntiles": int(tile_base[-1]), "win_bins": win_bins, "segs": segs,
    }


def _core_split(segs):
    """Per class: contiguous split across cores; returns per-core lists and
    padded Gmax."""
    out = {}
    for c in CLASSES:
        cl = segs[c]
        G = len(cl)
        per = []
        for ci in range(NCORES):
            lo = (G * ci) // NCORES
            hi = (G * (ci + 1)) // NCORES
            per.append(cl[lo:hi])
        Gmax = max(1, max(len(p) for p in per))
        out[c] = (per, Gmax)
    return out


def _build_core_inputs(split, feats_all):
    """Per-core input dict: per class wave-ordered feats streams + one-hot."""
    maps = [dict() for _ in range(NCORES)]
    meta = {c: [] for c in CLASSES}
    zrow = feats_all.shape[0] - 1                # zero sentinel tile
    for c in CLASSES:
        per, Gmax = split[c]
        for ci in range(NCORES):
            segs = per[ci]
            tid = np.full((Gmax, c), zrow, np.int64)
            wins = np.full(Gmax, -1, np.int64)
            for j, (w, t0) in enumerate(segs):
                tid[j] = np.arange(t0, t0 + c)
                wins[j] = w
            fc = feats_all[tid.reshape(-1)]       # [Gmax*c, 128, C2]
            fc = np.ascontiguousarray(
                fc.transpose(1, 0, 2).reshape(128, Gmax * c * C2))
            maps[ci][f"feats{c}"] = fc
            meta[c].append(wins)
    onehot = (np.arange(128)[:, None] // 8 == np.arange(W)[None, :])
    oh = np.ascontiguousarray(onehot.astype(FP8E3))
    for ci in range(NCORES):
        maps[ci]["onehot"] = oh
    return maps, meta


def _wave_plan(Gmax_by_class):
    """Emission order: one class-2 wave first, all class-4, rest of class-2.
    Returns list of (class, wave_idx, nseg_in_wave) and per-class wave count."""
    nwv = {c: (Gmax_by_class[c] + WAVE - 1) // WAVE for c in CLASSES}
    order = []

    def cw(c, w):
        ns = min(WAVE, Gmax_by_class[c] - w * WAVE)
        order.append((c, w, ns))

    if nwv[2] > 0:
        cw(2, 0)
    for w in range(nwv[4]):
        cw(4, w)
    for w in range(1, nwv[2]):
        cw(2, w)
    return order, nwv


def _build_bass(shape_key):
    """shape_key: tuple of (cls, Gmax) pairs, CLASSES order."""
    if shape_key in _BUILD_CACHE:
        return _BUILD_CACHE[shape_key]
    from concourse import bass, mybir, tile, bacc

    Gmax_by_class = dict(shape_key)
    nc = bacc.Bacc()
    params = {}
    for c, Gmax in shape_key:
        params[f"feats{c}"] = nc.declare_dram_parameter(
            f"feats{c}", [128, Gmax * c * C2], mybir.dt.float8e3,
            isOutput=False)
    order, nwv = _wave_plan(Gmax_by_class)
    for c in CLASSES:
        if nwv[c]:
            params[f"out{c}"] = nc.declare_dram_parameter(
                f"out{c}", [80, nwv[c] * STC], mybir.dt.bfloat16,
                isOutput=True)
    params["onehot"] = nc.declare_dram_parameter(
        "onehot", [128, W], mybir.dt.float8e3, isOutput=False)
    FW_COLS = 9 * CLASSES[0] * C2       # fixed sub-DMA tile size (class-4)

    with tile.TileContext(nc) as tc:
        with tc.tile_pool(name="fstream", bufs=9) as fpool, \
             tc.tile_pool(name="stage", bufs=4) as spool, \
             tc.tile_pool(name="const", bufs=1) as cpool, \
             tc.tile_pool(name="psum", bufs=2, space="PSUM") as psum_pool:
            oh_t = cpool.tile([128, W], mybir.dt.float8e3, tag="oh")
            nc.sync.dma_start(oh_t[:], params["onehot"][:, :])
            for wi, (c, wv, NW) in enumerate(order):
                # sub-DMA per third (s-major layout). Thirds 0-1 on the
                # sync HW-DGE ring, third 2 on gpsimd's SWDGE ring so the
                # two descriptor rings pace independently (the 8 HWDGE
                # semaphore lanes otherwise serialize trigger pacing).
                # per-third sub-DMAs on the sync ring (measured optimum:
                # whole-wave DMAs regress — engines fair-share across
                # outstanding transfers so big DMAs complete late; 9/wave
                # regresses on trigger overhead). Wave 0's thirds go out
                # on three parallel queues for a fast first MM.
                fts = []
                for t in range(NBANK):
                    ng = max(0, min(9, NW - t * 9))
                    ft = fpool.tile([128, 9 * CLASSES[0] * C2],
                                    mybir.dt.float8e3, tag=f"f{t}")
                    if ng:
                        a = (wv * WAVE + t * 9) * c * C2
                        eng = (nc.sync, nc.scalar, nc.gpsimd)[t] \
                            if wi == 0 else nc.sync
                        eng.dma_start(ft[:, :ng * c * C2],
                                      params[f"feats{c}"][:, a:a + ng * c * C2])
                    fts.append((ft, ng))
                mega = psum_pool.tile([80, NBANK * 512], mybir.dt.float32,
                                      tag="ps")
                # chains: triple-interleaved across column groups,
                # pairwise-distinct banks (bank = (g+t)%3)
                for t in range(NBANK):
                    ft, ng = fts[t]
                    for s in range(NSLOT):
                        for k in range(c):
                            for g in range(NCG):
                                pos = g * NSLOT + s
                                if pos >= ng:
                                    continue
                                b = (g + t) % NBANK
                                col = (pos * c + k) * C2
                                nc.tensor.matmul(
                                    out=mega[g * 32:g * 32 + W,
                                             b * 512 + s * C2:
                                             b * 512 + s * C2 + C2],
                                    lhsT=oh_t[:],
                                    rhs=ft[:, col:col + C2],
                                    start=(k == 0), stop=(k == c - 1))
                # PSUM -> SBUF (bf16), bank-major strided. ONE op covering
                # all 80 partitions (engine cost scales with free size only;
                # dead rows 16-31/48-63 copy garbage the host ignores) —
                # three per-colgroup ops get serialized by the scheduler's
                # transitive dep chaining and cost 3x. Alternate engines.
                st = spool.tile([80, STC], mybir.dt.bfloat16, tag="st")
                sub_m = mega[0:80, :]
                sub_s = st[0:80, :]
                src = bass.AP(sub_m.tensor, sub_m.offset,
                              [sub_m.ap[0], [512, NBANK], [1, NSLOT * C2]])
                dst = bass.AP(sub_s.tensor, sub_s.offset,
                              [sub_s.ap[0], [NSLOT * C2, NBANK],
                               [1, NSLOT * C2]])
                nc.vector.tensor_scalar_add(dst, src, 0.0)
                # ONE out-DMA per wave on gpsimd's SWDGE ring (80 rows
                # incl. dead ones the host ignores): 10 triggers instead
                # of 30 shortens the end-of-stream trigger serialization.
                # The very last wave's out goes on sync (input is done by
                # then, so no lane perturbation) letting gpsimd's slow
                # SWDGE DRAIN start a wave earlier and overlap the tail.
                oeng = nc.sync if wi == len(order) - 1 else nc.gpsimd
                oeng.dma_start(
                    params[f"out{c}"][:, wv * STC:(wv + 1) * STC],
                    st[:, :])
    nc.finalize()
    _BUILD_CACHE[shape_key] = nc
    return nc


def run_scheduled(x, flat, trace=False, trace_cores=None):
    """Core pipeline given precomputed flat bins; returns (grid, results)."""
    from concourse.bass_utils import run_bass_kernel_spmd

    plan = _plan(flat)
    xq = np.ascontiguousarray(x.reshape(-1, C)).astype(FP8E3)
    # global canonical tile store (+1 zero sentinel row)
    feats_all = np.zeros((plan["ntiles"] + 1, 128, C2), FP8E3)
    fview = feats_all.reshape(plan["ntiles"] + 1, 128, 2, C)
    fview[plan["gtile"], plan["lane"], plan["half"]] = xq[plan["pidx"]]

    split = _core_split(plan["segs"])
    shape_key = tuple((c, split[c][1]) for c in CLASSES)
    maps, meta = _build_core_inputs(split, feats_all)
    nc = _build_bass(shape_key)
    res = run_bass_kernel_spmd(nc, maps, core_ids=list(range(NCORES)),
                               trace=trace, trace_cores=trace_cores)

    Gmax_by_class = dict(shape_key)
    order, nwv = _wave_plan(Gmax_by_class)
    win_bins = plan["win_bins"]
    grid = np.zeros((NBINS, C), np.float32)
    for c in CLASSES:
        if not nwv[c]:
            continue
        for ci in range(NCORES):
            out = np.asarray(res.results[ci][f"out{c}"], np.float32)
            out = out.reshape(80, nwv[c], NBANK * NSLOT, C2)
            wins = meta[c][ci]
            live = np.nonzero(wins >= 0)[0]
            if not len(live):
                continue
            j = live
            wv, rem = j // WAVE, j % WAVE
            t, gp = rem // 9, rem % 9
            g, s = gp // NSLOT, gp % NSLOT
            blk = ((g + t) % NBANK) * NSLOT + s
            # vals[j] = out[g*32:g*32+16, wv, blk, :]
            rows = (g[:, None] * 32 + np.arange(W)[None, :])  # [nj, 16]
            vals = out[rows, wv[:, None], blk[:, None], :]    # [nj, 16, C2]
            vals = vals[..., :C] + vals[..., C:]              # [nj, 16, C]
            bins = win_bins[wins[j]]                          # [nj, 16]
            m = bins >= 0
            np.add.at(grid, bins[m], vals[m])
    return grid, res


def kernel(x, camera2lidar_rots, camera2lidar_trans, intrins, post_rots,
           post_trans, extra_rots, extra_trans):
    x = np.asarray(x, np.float32)
    B, N = x.shape[0], x.shape[1]
    assert (B, N) == (1, 6) and x.shape[2:] == (D, FH, FW, C), x.shape

    geom = _get_geometry(
        np.asarray(camera2lidar_rots, np.float32),
        np.asarray(camera2lidar_trans, np.float32),
        np.asarray(intrins, np.float32),
        np.asarray(post_rots, np.float32),
        np.asarray(post_trans, np.float32),
        np.asarray(extra_rots, np.float32),
        np.asarray(extra_trans, np.float32),
    )
    flat = _flat_bins(geom)[0]          # [Np]
    grid, _ = run_scheduled(x, flat)
    outp = grid.reshape(NXg, NYg, C).transpose(2, 0, 1)[None]  # [1,C,NX,NY]
    return np.ascontiguousarray(outp)
